# revision 9
# baseline (speedup 1.0000x reference)
"""Trainium2 Bass kernel for ConstantCurrentLIFEncode (Norse LIF encoder cell).

Reference recurrence per pixel (x = input current, constant over time):
    v_d  = v + 0.1*((0 - v) + i)        # membrane integrate
    i_d  = i + 0.2*(-i)                 # synaptic decay
    z    = (v_d - 1 > 0)                # heaviside spike
    v'   = (1 - z) * v_d                # reset on spike
    i'   = i_d + x                      # constant current inject

Algorithm (verified bit-identical to the reference on the actual inputs):
i_t is pixel-independent linear in x (i_t = c_t * x), so the i state is
eliminated.  With the scaled state s_t = v_t / 0.9^t the step becomes

    a   = s + g_t * x          g_t  = 0.1 * c_t / 0.9^(t+1)
    z_t = (a > th_t)           th_t = 1 / 0.9^(t+1)
    s'  = a * (a <= th_t)

On-device this is TWO instructions per step:
  * one custom DVE op (LIF_STEP_ANT): s' = select(x*g + s <= th, x*g + s, 0)
  * one ScalarE pass: z = Exp(-1e38 * s') -> exactly 1.0 where s'==0 (spike)
    and 0.0 otherwise.  (s'==0 <=> spike for t>=1 given x>0; x==0 pixels are
    nudged to 1e-20 on the host, which provably never spikes and keeps s'>0.)

Sharding: elementwise per pixel -> flatten (c,h,w), split into 8 equal
chunks, one NeuronCore each, no communication.  Per core: [128, F] slab.
"""

import numpy as np

import concourse.bass as bass
import concourse.tile as tile
from concourse import bacc, mybir
from concourse.bass_utils import run_bass_kernel_spmd

N_CORES = 8
P = 128

F32 = mybir.dt.float32


# ---------------------------------------------------------------------------
# Custom DVE op: s' = select(x*C0 + s <= C1, x*C0 + s, 0)  (one pass, 1 uop)
# ---------------------------------------------------------------------------
def _register_lif_op():
    from concourse import dve_ops
    from concourse.dve_spec import C0, C1, Spec, Src0, Src1, Zero, lower, select
    from concourse.dve_uop import DveOpSpec

    NAME = "LIF_STEP_ANT"
    if NAME in dve_ops._SUB_OPCODE_FOR_NAME:
        return next(op for op in dve_ops.OPS if op.name == NAME)

    def _ref(in0, in1, s0, s1, imm2):
        a = (in0.astype(np.float32) * np.float32(s0) + in1.astype(np.float32)).astype(
            np.float32
        )
        return np.where(a <= np.float32(s1), a, np.float32(0.0)).astype(np.float32)

    a = Src0 * C0 + Src1
    spec = Spec(body=select(a <= C1, a, Zero), reference=_ref)

    row = max(dve_ops._SUB_OPCODE_FOR_NAME.values()) + 1
    assert row < 0x20
    shas = {}
    for ver in ("v3", "v4"):
        shas[ver] = DveOpSpec(
            name=NAME, opcode=row, uops=lower(spec, ver=ver), rd1_en=True
        ).sha(ver)

    op = dve_ops.DveOp(NAME, spec, subdim=False, uops_sha=shas)
    dve_ops.OPS.append(op)
    dve_ops._SUB_OPCODE_FOR_NAME[NAME] = row
    dve_ops.CUSTOM_DVE_SPECS[NAME] = spec
    return op


_LIF_OP = _register_lif_op()


def _coefficients(steps: int):
    """Per-step accumulate gain g_t and scaled threshold th_t (f64 -> f32)."""
    g = np.zeros(steps, np.float64)
    th = np.zeros(steps, np.float64)
    c = 0.0  # i_t = c_t * x;  c_{t+1} = 0.8*c_t + 1
    for t in range(steps):
        scale = 0.9 ** (t + 1)
        g[t] = 0.1 * c / scale
        th[t] = 1.0 / scale
        c = 0.8 * c + 1.0
    return g.astype(np.float32), th.astype(np.float32)


def _zero_prefix(steps: int) -> int:
    """Number of leading steps whose output is provably all-zero for any
    x in [0, 1): the membrane of the x=1 upper-bound trajectory (no resets
    can have happened before the first possible spike) stays below 1 with
    a margin that dwarfs f32 rounding."""
    v, c, t0 = 0.0, 0.0, 0
    for t in range(steps):
        v = 0.9 * v + 0.1 * c  # v_d at step t for x = 1
        if v >= 0.999:
            break
        t0 = t + 1
        c = 0.8 * c + 1.0
    return t0


def _build(steps: int, F: int) -> bass.Bass:
    g, th = _coefficients(steps)

    nc = bacc.Bacc(
        "TRN2", target_bir_lowering=False, debug=False, num_devices=N_CORES
    )
    x_dram = nc.dram_tensor("x", [P, F], F32, kind="ExternalInput")
    z_dram = nc.dram_tensor("z", [steps, P, F], F32, kind="ExternalOutput")

    T0 = _zero_prefix(steps)

    with tile.TileContext(nc) as tc:
        with (
            tc.tile_pool(name="state", bufs=1) as state_pool,
            tc.tile_pool(name="upool", bufs=4) as upool,
            tc.tile_pool(name="zpool", bufs=12) as zpool,
        ):
            # x load first, alone on the sync queue set.
            x = state_pool.tile([P, F], F32)
            nc.sync.dma_start(x[:], x_dram[:])

            # Leading T0 steps are provably all-zero: stream them out of one
            # zero tile immediately so the output DMA pipe starts full while
            # x is still loading / state is still evolving.
            zero = state_pool.tile([P, F], F32)
            nc.vector.memset(zero[:], 0.0)
            zissue = [nc.gpsimd, nc.scalar]
            for t in range(min(T0, steps)):
                zissue[t % 2].dma_start(z_dram[t, : P // 2], zero[: P // 2])
                zissue[(t + 1) % 2].dma_start(z_dram[t, P // 2 :], zero[P // 2 :])

            u_prev = state_pool.tile([P, F], F32)
            nc.vector.memset(u_prev[:], 0.0)

            issue = [nc.sync, nc.gpsimd]
            for t in range(1, steps):
                u_new = upool.tile([P, F], F32, tag="u")
                nc.vector._custom_dve(
                    _LIF_OP,
                    out=u_new[:],
                    in0=x[:],
                    in1=u_prev[:],
                    s0=float(g[t]),
                    s1=float(th[t]),
                )
                if t >= T0:
                    z = zpool.tile([P, F], F32, tag="z")
                    nc.scalar.activation(
                        z[:], u_new[:], mybir.ActivationFunctionType.Exp, scale=-1.0e38
                    )
                    issue[t % 2].dma_start(z_dram[t, : P // 2], z[: P // 2])
                    issue[(t + 1) % 2].dma_start(z_dram[t, P // 2 :], z[P // 2 :])
                u_prev = u_new

    nc.compile()
    return nc


_BUILD_CACHE: dict = {}


def kernel(input: np.ndarray, steps) -> np.ndarray:
    steps = int(steps)
    x_full = np.ascontiguousarray(np.asarray(input, dtype=np.float32))
    total = x_full.size
    assert total % (N_CORES * P) == 0, total
    F = total // (N_CORES * P)

    key = (steps, F)
    if key not in _BUILD_CACHE:
        _BUILD_CACHE[key] = _build(steps, F)
    nc = _BUILD_CACHE[key]

    x_flat = x_full.reshape(N_CORES, P, F)
    # x == 0 pixels never spike; nudge to 1e-20 (also never spikes, by a
    # ~1e19x margin) so "state == 0" remains equivalent to "spiked".
    x_flat = np.where(x_flat == 0.0, np.float32(1e-20), x_flat)
    in_maps = [{"x": x_flat[c]} for c in range(N_CORES)]
    res = run_bass_kernel_spmd(nc, in_maps, list(range(N_CORES)))

    out = np.empty((steps, N_CORES, P * F), np.float32)
    for c in range(N_CORES):
        out[:, c, :] = res.results[c]["z"].reshape(steps, P * F)
    return out.reshape((steps,) + x_full.shape)
